# revision 1
# baseline (speedup 1.0000x reference)
"""Trainium2 Bass kernel for nn_CINLayer: out[b,d,o] = sum_{n,m} x[b,d,n]*y[b,d,m]*W[o,n*M+m].

Strategy (8-core data parallel over batch):
  Per sample s, out[o,s] = sum_k Wl[k,o] * Z[k,s] with Z[k,s] = x[s,n(k)]*y[s,m(k)].
  The contraction k (1600 products) is split into 13 chunks of 128 rows whose
  row->(n,m) mapping is chosen so each chunk's X-factor tile is a single
  DVE stream_shuffle of a host-staged interleaved layout Xil (per-quadrant
  lane-broadcast), and the Y-factor tiles are host-staged replicated layouts.
  Z chunks are built as one shuffle + one fp16 tensor_mul, then fed as the
  moving operand of fp16 matmuls accumulating out^T[o, s] in PSUM
  (o split 128+72, s tiles of 512).

  Chunk row mapping (r = 32j + r', j=quadrant):
    Part A (c<10):  (n, m) = (4c + j, r')          for r' < 32
    Part B (cb=c-10<3): r' = 8a + m''; (n, m) = (16cb + 4a + j, 32 + m'')
  Host layouts:
    Xil[32j + i]  = xT[4i + j]   (i<10, else 0)
    YrepA[p]      = yT[p % 32]
    YrepB[p]      = yT[32 + p % 8]
  Shuffle masks: A: mask[r'] = c ; B: mask[r'] = 4*cb + r'//8.
  W rows with n >= 40 (part B overhang) are zeroed on host.
"""

import numpy as np

BS, DIM, N, M, O = 2048, 32, 40, 40, 200
NCORES = 8
S_PER_CORE = BS * DIM // NCORES  # 8192
S_TILE = 512
N_STILES_FULL = S_PER_CORE // S_TILE  # 16
NCHUNKS = 13  # 10 part-A + 3 part-B
F16 = np.float16

# chunks whose Z-multiply runs on GPSIMD instead of DVE. GPSIMD's tensor_mul
# is ~9x slower per op than DVE's, but running a few there in parallel with
# the DVE shuffle/mul stream measured fastest (190us vs 214us all-DVE).
GPSIMD_MULS = frozenset({2, 4, 6, 9, 11})


def _chunk_row_to_nm(c: int, r: int):
    """Global chunk c (0..12), row r (0..127) -> (n, m) or None (zero pad)."""
    j, rp = divmod(r, 32)
    if c < 10:
        return 4 * c + j, rp
    cb = c - 10
    a, mpp = divmod(rp, 8)
    n = 16 * cb + 4 * a + j
    if n >= N:
        return None
    return n, 32 + mpp


def _shuffle_mask(c: int):
    if c < 10:
        return [c] * 32
    cb = c - 10
    return [4 * cb + (rp // 8) for rp in range(32)]


def _stage_w(W: np.ndarray) -> np.ndarray:
    """W [O, N*M] f32 -> wl [128, NCHUNKS, O] f16 (lhsT layout per chunk)."""
    Wr = W.reshape(O, N, M)
    wl = np.zeros((128, NCHUNKS, O), dtype=F16)
    for c in range(NCHUNKS):
        for r in range(128):
            nm = _chunk_row_to_nm(c, r)
            if nm is not None:
                wl[r, c, :] = Wr[:, nm[0], nm[1]].astype(F16)
    return wl


def _stage_core_inputs(x_flat: np.ndarray, y_flat: np.ndarray):
    """x_flat, y_flat [S_PER_CORE, 40] f32 -> xil, yrepa, yrepb [128, S] f16."""
    xT = np.ascontiguousarray(x_flat.T).astype(F16)  # [40, S]
    yT = np.ascontiguousarray(y_flat.T).astype(F16)  # [40, S]
    s = xT.shape[1]
    xil = np.zeros((128, s), dtype=F16)
    for p in range(128):
        j, i = divmod(p, 32)[0], p % 32
        if i < 10:
            xil[p] = xT[4 * i + j]
    yrepa = yT[np.arange(128) % 32]
    yrepb = yT[32 + (np.arange(128) % 8)]
    return xil, np.ascontiguousarray(yrepa), np.ascontiguousarray(yrepb)


def build_nc(n_stiles: int = N_STILES_FULL, debug: bool = False):
    """Build the per-core Bass/Tile module. Returns (nc, names dict)."""
    import concourse.bass as bass
    import concourse.tile as tile
    from concourse import bacc, mybir
    from concourse.tile_rust import add_dep_helper

    f16 = mybir.dt.float16
    f32 = mybir.dt.float32
    s_len = n_stiles * S_TILE

    nc = bacc.Bacc("TRN2", target_bir_lowering=False, debug=debug)

    xil_d = nc.dram_tensor("xil", [128, s_len], f16, kind="ExternalInput")
    ya_d = nc.dram_tensor("yrepa", [128, s_len], f16, kind="ExternalInput")
    yb_d = nc.dram_tensor("yrepb", [128, s_len], f16, kind="ExternalInput")
    wl_d = nc.dram_tensor("wl", [128, NCHUNKS, O], f16, kind="ExternalInput")
    out_d = nc.dram_tensor("outt", [O, s_len], f16, kind="ExternalOutput")

    with tile.TileContext(nc) as tc:
        with (
            tc.tile_pool(name="wpool", bufs=1) as wpool,
            tc.tile_pool(name="inp", bufs=4) as inp,
            tc.tile_pool(name="xe", bufs=8) as xep,
            tc.tile_pool(name="zp", bufs=8) as zp,
            tc.tile_pool(name="outp", bufs=4) as outp,
            tc.tile_pool(name="ps", bufs=2, space=bass.MemorySpace.PSUM) as psp,
        ):
            wl_sb = wpool.tile([128, NCHUNKS, O], f16)
            nc.sync.dma_start(wl_sb[:], wl_d[:])

            # Paired s-tiles: each shuffle/mul covers 1024 samples (two matmul
            # tiles) to halve DVE op count and PE supply-wait events; the four
            # PSUM accumulation chains use exactly 8 banks at bufs=2.
            W2 = 2 * S_TILE
            for t2 in range(n_stiles // 2):
                sl2 = bass.ts(t2, W2)
                xil_t = inp.tile([128, W2], f16)
                nc.sync.dma_start(xil_t[:], xil_d[:, sl2])
                ya_t = inp.tile([128, W2], f16)
                nc.sync.dma_start(ya_t[:], ya_d[:, sl2])
                yb_t = inp.tile([128, W2], f16)
                nc.sync.dma_start(yb_t[:], yb_d[:, sl2])

                psA0 = psp.tile([128, S_TILE], f32, tag="psA0")
                psB0 = psp.tile([72, S_TILE], f32, tag="psB0")
                psA1 = psp.tile([128, S_TILE], f32, tag="psA1")
                psB1 = psp.tile([72, S_TILE], f32, tag="psB1")
                ps = [psA0, psB0, psA1, psB1]
                for c in range(NCHUNKS):
                    xe = xep.tile([128, W2], f16, tag="xe")
                    nc.vector.stream_shuffle(xe[:], xil_t[:], _shuffle_mask(c))
                    z = zp.tile([128, W2], f16)
                    yt = ya_t if c < 10 else yb_t
                    eng = nc.gpsimd if c in GPSIMD_MULS else nc.vector
                    eng.tensor_mul(z[:], yt[:], xe[:])
                    first, last = c == 0, c == NCHUNKS - 1
                    for h in range(2):
                        zh = z[:, h * S_TILE : (h + 1) * S_TILE]
                        nc.tensor.matmul(
                            ps[2 * h][:], wl_sb[:, c, 0:128], zh,
                            start=first, stop=last,
                        )
                        nc.tensor.matmul(
                            ps[2 * h + 1][:], wl_sb[:, c, 128:200], zh,
                            start=first, stop=last,
                        )

                for h in range(2):
                    sl = bass.ts(2 * t2 + h, S_TILE)
                    oA = outp.tile([128, S_TILE], f16, tag="oA")
                    nc.scalar.copy(oA[:], ps[2 * h][:])
                    oB = outp.tile([72, S_TILE], f16, tag="oB")
                    nc.scalar.copy(oB[:], ps[2 * h + 1][:])
                    nc.scalar.dma_start(out_d[0:128, sl], oA[:])
                    nc.scalar.dma_start(out_d[128:200, sl], oB[:])

    nc.compile()
    return nc


def kernel(x: np.ndarray, y: np.ndarray, W: np.ndarray) -> np.ndarray:
    from concourse.bass_utils import run_bass_kernel_spmd

    assert x.shape == (BS, DIM, N) and y.shape == (BS, DIM, M)
    assert W.shape == (O, N * M)

    wl = _stage_w(W)
    x_cores = x.reshape(NCORES, S_PER_CORE, N)
    y_cores = y.reshape(NCORES, S_PER_CORE, M)

    in_maps = []
    for i in range(NCORES):
        xil, yrepa, yrepb = _stage_core_inputs(x_cores[i], y_cores[i])
        in_maps.append({"xil": xil, "yrepa": yrepa, "yrepb": yrepb, "wl": wl})

    nc = build_nc()
    res = run_bass_kernel_spmd(nc, in_maps, core_ids=list(range(NCORES)))

    outs = []
    for i in range(NCORES):
        outt = res.results[i]["outt"]  # [O, S_PER_CORE] f16
        outs.append(outt.T.astype(np.float32))  # [S_PER_CORE, O]
    return np.concatenate(outs, axis=0).reshape(BS, DIM, O)


if __name__ == "__main__":
    xs = np.random.randn(BS, DIM, N).astype(np.float32)
    ys = np.random.randn(BS, DIM, M).astype(np.float32)
    Ws = (np.random.randn(O, N * M) * (1.0 / np.sqrt(N * M))).astype(np.float32)
    out = kernel(xs, ys, Ws)
    print(out.shape, out.dtype)



# revision 2
# speedup vs baseline: 1.5090x; 1.5090x over previous
"""Trainium2 Bass kernel for nn_CINLayer: out[b,d,o] = sum_{n,m} x[b,d,n]*y[b,d,m]*W[o,n*M+m].

Strategy (8-core data parallel over batch):
  Per sample s, out[o,s] = sum_k Wl[k,o] * Z[k,s] with Z[k,s] = x[s,n(k)]*y[s,m(k)].
  The 1600-term contraction is split into 13 chunks of 128 rows whose
  row->(n,m) mapping is chosen so that BOTH factor tiles of every chunk are
  host-staged replicated layouts (no on-device cross-partition movement):

    chunks 0-4  (n 0..15):  n = r//8,      m = 8c + r%8
    chunks 5-9  (n 16..31): n = 16 + r//8, m = 8(c-5) + r%8
    chunks 10-12 (n 32..39): n = 32 + r//16, m = 16(c-10) + r%16  (m>=40 -> W row zeroed)

  This needs only 11 distinct [128, S] factor tiles per core:
    xg0[r]=xT[r//8], xg1[r]=xT[16+r//8], xg2[r]=xT[32+r//16],
    y0..y4[r]=yT[8q+r%8], yg2_0..2[r]=yT[(16q+r%16)%40]
  staged as one contiguous per-s-tile DRAM block so each loop iteration is a
  single large DMA. Per chunk the DVE does one fp16 tensor_mul (2x mode) and
  the PE accumulates out^T[o,s] (o split 128+72, s tiles of 512) in PSUM.
"""

import numpy as np

BS, DIM, N, M, O = 2048, 32, 40, 40, 200
NCORES = 8
S_PER_CORE = BS * DIM // NCORES  # 8192
S_TILE = 512
N_STILES_FULL = S_PER_CORE // S_TILE  # 16
NCHUNKS = 13
NTILES = 11  # 3 x-factor + 8 y-factor layouts
W2 = 2 * S_TILE  # samples per loop iteration (paired s-tiles)
F16 = np.float16


def _chunk_row_to_nm(c: int, r: int):
    """Chunk c (0..12), row r (0..127) -> (n, m) or None (zero pad)."""
    if c < 5:
        return r // 8, 8 * c + r % 8
    if c < 10:
        return 16 + r // 8, 8 * (c - 5) + r % 8
    m = 16 * (c - 10) + r % 16
    if m >= M:
        return None
    return 32 + r // 16, m


def _chunk_srcs(c: int):
    """Chunk c -> (x tile idx, y tile idx) in the staged block."""
    if c < 5:
        return 0, 3 + c
    if c < 10:
        return 1, 3 + (c - 5)
    return 2, 8 + (c - 10)


def _stage_w(W: np.ndarray) -> np.ndarray:
    """W [O, N*M] f32 -> wl [128, NCHUNKS, O] f16 (lhsT layout per chunk)."""
    Wr = W.reshape(O, N, M)
    wl = np.zeros((128, NCHUNKS, O), dtype=F16)
    for c in range(NCHUNKS):
        for r in range(128):
            nm = _chunk_row_to_nm(c, r)
            if nm is not None:
                wl[r, c, :] = Wr[:, nm[0], nm[1]].astype(F16)
    return wl


def _stage_core_inputs(x_flat: np.ndarray, y_flat: np.ndarray) -> np.ndarray:
    """x_flat, y_flat [S_PER_CORE, 40] f32 -> xy [128, n_t2, NTILES, W2] f16.

    Factor tiles interleaved per t2-iteration so each iteration's input is one
    fully contiguous per-partition DMA."""
    xT = np.ascontiguousarray(x_flat.T).astype(F16)  # [40, S]
    yT = np.ascontiguousarray(y_flat.T).astype(F16)  # [40, S]
    r = np.arange(128)
    tiles = [
        xT[r // 8],
        xT[16 + r // 8],
        xT[32 + r // 16],
        yT[0 + r % 8],
        yT[8 + r % 8],
        yT[16 + r % 8],
        yT[24 + r % 8],
        yT[32 + r % 8],
        yT[(0 + r % 16) % M],
        yT[(16 + r % 16) % M],
        yT[(32 + r % 16) % M],
    ]
    xy = np.stack(tiles, axis=1)  # [128, NTILES, S]
    s = xT.shape[1]
    xy = xy.reshape(128, NTILES, s // W2, W2).transpose(0, 2, 1, 3)
    return np.ascontiguousarray(xy)  # [128, n_t2, NTILES, W2]


def build_nc(n_stiles: int = N_STILES_FULL, debug: bool = False):
    """Build the per-core Bass/Tile module. Returns nc."""
    import concourse.bass as bass
    import concourse.tile as tile
    from concourse import bacc, mybir

    f16 = mybir.dt.float16
    f32 = mybir.dt.float32
    s_len = n_stiles * S_TILE
    n_t2 = n_stiles // 2

    nc = bacc.Bacc("TRN2", target_bir_lowering=False, debug=debug)

    xy_d = nc.dram_tensor("xy", [128, n_t2, NTILES, W2], f16, kind="ExternalInput")
    wl_d = nc.dram_tensor("wl", [128, NCHUNKS, O], f16, kind="ExternalInput")
    out_d = nc.dram_tensor("outt", [O, s_len], f16, kind="ExternalOutput")

    with tile.TileContext(nc) as tc:
        with (
            tc.tile_pool(name="wpool", bufs=1) as wpool,
            tc.tile_pool(name="inp", bufs=3) as inp,
            tc.tile_pool(name="zp", bufs=8) as zp,
            tc.tile_pool(name="outp", bufs=2) as outp,
            tc.tile_pool(name="ps", bufs=2, space=bass.MemorySpace.PSUM) as psp,
        ):
            wl_sb = wpool.tile([128, NCHUNKS, O], f16)
            nc.sync.dma_start(wl_sb[:], wl_d[:])

            for t2 in range(n_t2):
                sl2 = bass.ts(t2, W2)
                xy_t = inp.tile([128, NTILES, W2], f16)
                nc.sync.dma_start(xy_t[:], xy_d[:, t2, :, :])

                psA0 = psp.tile([128, S_TILE], f32, tag="psA0")
                psB0 = psp.tile([72, S_TILE], f32, tag="psB0")
                psA1 = psp.tile([128, S_TILE], f32, tag="psA1")
                psB1 = psp.tile([72, S_TILE], f32, tag="psB1")
                ps = [psA0, psB0, psA1, psB1]
                for c in range(NCHUNKS):
                    xi, yi = _chunk_srcs(c)
                    z = zp.tile([128, W2], f16)
                    nc.vector.tensor_mul(z[:], xy_t[:, yi, :], xy_t[:, xi, :])
                    first, last = c == 0, c == NCHUNKS - 1
                    for h in range(2):
                        zh = z[:, h * S_TILE : (h + 1) * S_TILE]
                        nc.tensor.matmul(
                            ps[2 * h][:], wl_sb[:, c, 0:128], zh,
                            start=first, stop=last,
                        )
                        nc.tensor.matmul(
                            ps[2 * h + 1][:], wl_sb[:, c, 128:200], zh,
                            start=first, stop=last,
                        )

                oA = outp.tile([128, W2], f16, tag="oA")
                oB = outp.tile([72, W2], f16, tag="oB")
                for h in range(2):
                    hs = slice(h * S_TILE, (h + 1) * S_TILE)
                    nc.scalar.copy(oA[:, hs], ps[2 * h][:])
                    nc.scalar.copy(oB[:, hs], ps[2 * h + 1][:])
                nc.scalar.dma_start(out_d[0:128, sl2], oA[:])
                nc.scalar.dma_start(out_d[128:200, sl2], oB[:])

    nc.compile()
    return nc


def prepare(x: np.ndarray, y: np.ndarray, W: np.ndarray):
    """Stage full inputs -> (nc, per-core input maps)."""
    assert x.shape == (BS, DIM, N) and y.shape == (BS, DIM, M)
    assert W.shape == (O, N * M)

    wl = _stage_w(W)
    x_cores = x.reshape(NCORES, S_PER_CORE, N)
    y_cores = y.reshape(NCORES, S_PER_CORE, M)
    in_maps = []
    for i in range(NCORES):
        xy = _stage_core_inputs(x_cores[i], y_cores[i])
        in_maps.append({"xy": xy, "wl": wl})
    nc = build_nc()
    return nc, in_maps


def collect(res) -> np.ndarray:
    outs = []
    for i in range(NCORES):
        outt = res.results[i]["outt"]  # [O, S_PER_CORE] f16
        outs.append(outt.T.astype(np.float32))
    return np.concatenate(outs, axis=0).reshape(BS, DIM, O)


def kernel(x: np.ndarray, y: np.ndarray, W: np.ndarray) -> np.ndarray:
    from concourse.bass_utils import run_bass_kernel_spmd

    nc, in_maps = prepare(x, y, W)
    res = run_bass_kernel_spmd(nc, in_maps, core_ids=list(range(NCORES)))
    return collect(res)


if __name__ == "__main__":
    xs = np.random.randn(BS, DIM, N).astype(np.float32)
    ys = np.random.randn(BS, DIM, M).astype(np.float32)
    Ws = (np.random.randn(O, N * M) * (1.0 / np.sqrt(N * M))).astype(np.float32)
    out = kernel(xs, ys, Ws)
    print(out.shape, out.dtype)


# revision 4
# speedup vs baseline: 1.5569x; 1.0318x over previous
"""Trainium2 Bass kernel for nn_CINLayer: out[b,d,o] = sum_{n,m} x[b,d,n]*y[b,d,m]*W[o,n*M+m].

Strategy (8-core data parallel over batch):
  Per sample s, out[o,s] = sum_k Wl[k,o] * Z[k,s] with Z[k,s] = x[s,n(k)]*y[s,m(k)].
  The 1600-term contraction is split into 13 chunks of 128 rows whose
  row->(n,m) mapping is chosen so that BOTH factor tiles of every chunk are
  host-staged replicated layouts (no on-device cross-partition movement):

    chunks 0-4  (n 0..15):  n = r//8,      m = 8c + r%8
    chunks 5-9  (n 16..31): n = 16 + r//8, m = 8(c-5) + r%8
    chunks 10-12 (n 32..39): n = 32 + r//16, m = 16(c-10) + r%16  (m>=40 -> W row zeroed)

  This needs only 11 distinct [128, S] factor tiles per core:
    xg0[r]=xT[r//8], xg1[r]=xT[16+r//8], xg2[r]=xT[32+r//16],
    y0..y4[r]=yT[8q+r%8], yg2_0..2[r]=yT[(16q+r%16)%40]
  staged as one contiguous per-s-tile DRAM block so each loop iteration is a
  single large DMA. Per chunk the DVE does one fp16 tensor_mul (2x mode) and
  the PE accumulates out^T[o,s] (o split 128+72, s tiles of 512) in PSUM.
"""

import numpy as np

BS, DIM, N, M, O = 2048, 32, 40, 40, 200
NCORES = 8
S_PER_CORE = BS * DIM // NCORES  # 8192
S_TILE = 512
N_STILES_FULL = S_PER_CORE // S_TILE  # 16
NCHUNKS = 13
NTILES = 11  # 3 x-factor + 8 y-factor layouts
W2 = 2 * S_TILE  # samples per loop iteration (paired s-tiles)
F16 = np.float16


def _chunk_row_to_nm(c: int, r: int):
    """Chunk c (0..12), row r (0..127) -> (n, m) or None (zero pad)."""
    if c < 5:
        return r // 8, 8 * c + r % 8
    if c < 10:
        return 16 + r // 8, 8 * (c - 5) + r % 8
    m = 16 * (c - 10) + r % 16
    if m >= M:
        return None
    return 32 + r // 16, m


def _chunk_srcs(c: int):
    """Chunk c -> (x tile idx, y tile idx) in the staged block."""
    if c < 5:
        return 0, 3 + c
    if c < 10:
        return 1, 3 + (c - 5)
    return 2, 8 + (c - 10)


def _stage_w(W: np.ndarray) -> np.ndarray:
    """W [O, N*M] f32 -> wl [128, NCHUNKS, O] f16 (lhsT layout per chunk)."""
    Wr = W.reshape(O, N, M)
    wl = np.zeros((128, NCHUNKS, O), dtype=F16)
    for c in range(NCHUNKS):
        for r in range(128):
            nm = _chunk_row_to_nm(c, r)
            if nm is not None:
                wl[r, c, :] = Wr[:, nm[0], nm[1]].astype(F16)
    return wl


def _stage_core_inputs(x_flat: np.ndarray, y_flat: np.ndarray) -> np.ndarray:
    """x_flat, y_flat [S_PER_CORE, 40] f32 -> xy [128, n_t2, NTILES, W2] f16.

    Factor tiles interleaved per t2-iteration so each iteration's input is one
    fully contiguous per-partition DMA."""
    xT = np.ascontiguousarray(x_flat.T).astype(F16)  # [40, S]
    yT = np.ascontiguousarray(y_flat.T).astype(F16)  # [40, S]
    r = np.arange(128)
    tiles = [
        xT[r // 8],
        xT[16 + r // 8],
        xT[32 + r // 16],
        yT[0 + r % 8],
        yT[8 + r % 8],
        yT[16 + r % 8],
        yT[24 + r % 8],
        yT[32 + r % 8],
        yT[(0 + r % 16) % M],
        yT[(16 + r % 16) % M],
        yT[(32 + r % 16) % M],
    ]
    xy = np.stack(tiles, axis=1)  # [128, NTILES, S]
    s = xT.shape[1]
    xy = xy.reshape(128, NTILES, s // W2, W2).transpose(0, 2, 1, 3)
    return np.ascontiguousarray(xy)  # [128, n_t2, NTILES, W2]


def build_nc(n_stiles: int = N_STILES_FULL, debug: bool = False):
    """Build the per-core Bass/Tile module. Returns nc."""
    import concourse.bass as bass
    import concourse.tile as tile
    from concourse import bacc, mybir

    f16 = mybir.dt.float16
    f32 = mybir.dt.float32
    s_len = n_stiles * S_TILE
    n_t2 = n_stiles // 2

    nc = bacc.Bacc("TRN2", target_bir_lowering=False, debug=debug)

    xy_d = nc.dram_tensor("xy", [128, n_t2, NTILES, W2], f16, kind="ExternalInput")
    wl_d = nc.dram_tensor("wl", [128, NCHUNKS, O], f16, kind="ExternalInput")
    out_d = nc.dram_tensor("outt", [O, s_len], f16, kind="ExternalOutput")

    # tile DMA order for the first iteration: chunk c needs (x,y) tile pair
    # (0,3),(0,4)..(0,7),(1,3)..(1,7),(2,8),(2,9),(2,10) -> need-order below
    FIRST_ORDER = [0, 3, 4, 5, 6, 7, 1, 2, 8, 9, 10]
    N_WARM = 36

    with tile.TileContext(nc) as tc:
        with (
            tc.tile_pool(name="wpool", bufs=1) as wpool,
            tc.tile_pool(name="inp", bufs=3) as inp,
            tc.tile_pool(name="zp", bufs=8) as zp,
            tc.tile_pool(name="outp", bufs=2) as outp,
            tc.tile_pool(name="ps", bufs=2, space=bass.MemorySpace.PSUM) as psp,
        ):
            wl_sb = wpool.tile([128, NCHUNKS, O], f16)
            # scalar queue so wl streams concurrently with the sync-queue
            # xy tiles (it gates the first LDWEIGHTS)
            nc.scalar.dma_start(wl_sb[:], wl_d[:])

            # PE warmup: dummy matmuls while input DMAs land, so HAM has
            # un-throttled the clock (1.2->2.4 GHz) before the real stream
            warm_sb = wpool.tile([128, 128], f16)
            nc.gpsimd.memset(warm_sb[:], 0)
            warm_ps = psp.tile([128, S_TILE], f32, tag="psA0")
            for _ in range(N_WARM):
                nc.tensor.matmul(
                    warm_ps[:, 0:128], warm_sb[:], warm_sb[:],
                    start=True, stop=True,
                )

            for t2 in range(n_t2):
                sl2 = bass.ts(t2, W2)
                xy_t = inp.tile([128, NTILES, W2], f16)
                if t2 == 0:
                    for i in FIRST_ORDER:
                        nc.sync.dma_start(xy_t[:, i, :], xy_d[:, t2, i, :])
                else:
                    nc.sync.dma_start(xy_t[:], xy_d[:, t2, :, :])

                psA0 = psp.tile([128, S_TILE], f32, tag="psA0")
                psB0 = psp.tile([72, S_TILE], f32, tag="psB0")
                psA1 = psp.tile([128, S_TILE], f32, tag="psA1")
                psB1 = psp.tile([72, S_TILE], f32, tag="psB1")
                ps = [psA0, psB0, psA1, psB1]
                for c in range(NCHUNKS):
                    xi, yi = _chunk_srcs(c)
                    z = zp.tile([128, W2], f16)
                    nc.vector.tensor_mul(z[:], xy_t[:, yi, :], xy_t[:, xi, :])
                    first, last = c == 0, c == NCHUNKS - 1
                    for h in range(2):
                        zh = z[:, h * S_TILE : (h + 1) * S_TILE]
                        nc.tensor.matmul(
                            ps[2 * h][:], wl_sb[:, c, 0:128], zh,
                            start=first, stop=last,
                        )
                        nc.tensor.matmul(
                            ps[2 * h + 1][:], wl_sb[:, c, 128:200], zh,
                            start=first, stop=last,
                        )

                oA = outp.tile([128, W2], f16, tag="oA")
                oB = outp.tile([72, W2], f16, tag="oB")
                for h in range(2):
                    hs = slice(h * S_TILE, (h + 1) * S_TILE)
                    nc.scalar.copy(oA[:, hs], ps[2 * h][:])
                    nc.vector.tensor_copy(oB[:, hs], ps[2 * h + 1][:])
                nc.scalar.dma_start(out_d[0:128, sl2], oA[:])
                nc.sync.dma_start(out_d[128:200, sl2], oB[:])

    nc.compile()
    return nc


def prepare(x: np.ndarray, y: np.ndarray, W: np.ndarray):
    """Stage full inputs -> (nc, per-core input maps)."""
    assert x.shape == (BS, DIM, N) and y.shape == (BS, DIM, M)
    assert W.shape == (O, N * M)

    wl = _stage_w(W)
    x_cores = x.reshape(NCORES, S_PER_CORE, N)
    y_cores = y.reshape(NCORES, S_PER_CORE, M)
    in_maps = []
    for i in range(NCORES):
        xy = _stage_core_inputs(x_cores[i], y_cores[i])
        in_maps.append({"xy": xy, "wl": wl})
    nc = build_nc()
    return nc, in_maps


def collect(res) -> np.ndarray:
    outs = []
    for i in range(NCORES):
        outt = res.results[i]["outt"]  # [O, S_PER_CORE] f16
        outs.append(outt.T.astype(np.float32))
    return np.concatenate(outs, axis=0).reshape(BS, DIM, O)


def kernel(x: np.ndarray, y: np.ndarray, W: np.ndarray) -> np.ndarray:
    from concourse.bass_utils import run_bass_kernel_spmd

    nc, in_maps = prepare(x, y, W)
    res = run_bass_kernel_spmd(nc, in_maps, core_ids=list(range(NCORES)))
    return collect(res)


if __name__ == "__main__":
    xs = np.random.randn(BS, DIM, N).astype(np.float32)
    ys = np.random.randn(BS, DIM, M).astype(np.float32)
    Ws = (np.random.randn(O, N * M) * (1.0 / np.sqrt(N * M))).astype(np.float32)
    out = kernel(xs, ys, Ws)
    print(out.shape, out.dtype)
